# revision 28
# baseline (speedup 1.0000x reference)
"""Trainium2 Bass kernel for nn_AttentionHead (B=8, S=2048, DK=512).

Reference semantics (faithful to the source module, bugs included):
    qh = q @ Wq.T + bq            # [B, S, D]
    kh = k @ Wk.T + bk
    vh = v @ Wv.T + bv
    kr = kh.reshape(B, D, S)      # row-major REINTERPRET, not a transpose
    s  = (qh @ kr) * sqrt(D)      # source bug: multiplies by sqrt(D)
    a  = softmax(s, axis=2)
    out = a @ vh                  # [B, S, dk]

Sharding: data-parallel over batch - one batch element per NeuronCore,
8 cores, no collectives. Each core runs the same NEFF with its own shard.

Per-core dataflow (v2 - stage-pipelined projections):
  - q,k and Wq,Wk are PE-transposed in fp32r (scores need tf32 accuracy:
    the kept source bug multiplies scores by sqrt(D), making the softmax
    near-one-hot and argmax-sensitive; bf16 q/k fails the error budget).
  - The v path is bf16 end-to-end: v tiles are cast f32->bf16 on GpSimd
    (an otherwise idle engine) and transposed by the DMA XBAR in one
    [128,2048]->[128,16,128] shot per 4-tile group; Wv likewise (cast +
    one XBAR shot) - no PE transposes and no PSUM->SBUF staging copies
    on the whole v path.  vh only feeds the bf16 a@vh matmul, and out
    is a near-one-hot selection of vh rows, so bf16 projection inputs
    cost ~0.2% extra error against a 2e-2 budget.
  - All projection loops are software-pipelined one stage deep: the
    transposes (+DMA) of stage n+1 are emitted BEFORE the matmuls of
    stage n, so the PE-transpose -> ScalarE-staging-copy -> matmul
    chain's copy latency hides under the previous stage's matmuls.
    (The v1 kernel serialized T(n)->copy(n)->M(n) per iteration and
    lost ~300-500ns of PE time per iteration waiting on ScalarE.)
  - DMA head order: Wk tiles first (they gate the Wk transposes that
    gate everything), then bkb, then two k tile-groups; Wv/bvb are
    issued from inside kh stage 3, Wq/bqT from inside vh stage 2, so
    each weight lands ~2 stages before its consumers without delaying
    the k/v tile stream.  (v1 put 1MB of k tiles ahead of Wk and the
    first real PE work started at 14.6us; now ~6us.)
  - kr = kh.reshape(D, S) is produced with no data movement: the kh
    projection picks its stationary columns with a stride-4 AP packing
    done by the PSUM->SBUF staging copy (ScalarE), which permutes the
    PSUM partition order into kr's layout; the bias-add writes straight
    into the resident kr tile.
  - scores: s[i_part, j] accumulated in PSUM with fp32r (tf32) matmuls.
  - softmax: each 512-chunk PSUM is copied to SBUF (DVE) immediately so
    the bank frees fast; ONE ScalarE exp covers the whole 2048-wide row
    and its accumulator read IS the softmax denominator.
  - probs are transposed by the DMA XBAR (one [128,2048]->[128,16,128]
    shot per i-block) on the ACT HWDGE ring (the SP ring strands the
    transpose's tail descriptors until the next SP dispatch -> ~50%
    stale data).  The attention loop is software-pipelined 4 deep; the
    last two blocks transpose probs on the (then-idle) PE instead.
  - qh bias rides a DVE tensor_scalar_add (ScalarE in the qh phase is
    loaded with the qTc staging copies; DVE is idle there).
  - dummy bf16 matmuls at kernel start fill the initial DMA wait and
    warm the PE clock gate to 2.4 GHz before real work arrives.

Known dead ends (measured, do not retry): v-path PER-TILE XBAR (16
dispatches x ~1.1us fixed cost on ScalarE dwarf the PE saving - batch
4 tiles per shot instead); io bufs < 8; tensor_tensor_reduce with a
PSUM input (wedges the device); per-chunk XBAR splits; PE-transposing
block 13's probs; fp8/DoubleRow (accuracy: near-one-hot softmax is
argmax-sensitive, fp8 value paths exceed the 2e-2 budget); fp32 XBAR
transpose (HW caps 4-byte transposes at 64 output partitions).
"""

from contextlib import ExitStack

import numpy as np

import concourse.bacc as bacc
import concourse.mybir as mybir
import concourse.tile as tile
from concourse.bass_utils import run_bass_kernel_spmd
from concourse.masks import make_identity

AF = mybir.ActivationFunctionType
ALU = mybir.AluOpType
AX = mybir.AxisListType
F32 = mybir.dt.float32
F32R = mybir.dt.float32r
BF16 = mybir.dt.bfloat16

B, S, D = 8, 2048, 512
P = 128
NT_S = S // P          # 16 s-tiles (also j-tiles / i-blocks)
NT_D = D // P          # 4 d-tiles (also e-tiles)
NCH = S // 512         # 4 512-wide chunks of the sequence dim
SQRT_D = float(np.sqrt(np.float32(D)))

N_WARM = 12            # PE warm-up matmuls (fill DMA head, warm clock)
XBAR_PROBS = True      # probs transpose via DMA XBAR instead of PE


def build_nc():
    nc = bacc.Bacc("TRN2", target_bir_lowering=False, debug=False,
                   enable_asserts=False, num_devices=B)

    q = nc.dram_tensor("q", [S, D], F32, kind="ExternalInput").ap()
    k = nc.dram_tensor("k", [S, D], F32, kind="ExternalInput").ap()
    v = nc.dram_tensor("v", [S, D], F32, kind="ExternalInput").ap()
    Wq = nc.dram_tensor("Wq", [D, D], F32, kind="ExternalInput").ap()
    Wk = nc.dram_tensor("Wk", [D, D], F32, kind="ExternalInput").ap()
    Wv = nc.dram_tensor("Wv", [D, D], F32, kind="ExternalInput").ap()
    bq = nc.dram_tensor("bq", [D], F32, kind="ExternalInput").ap()
    bk = nc.dram_tensor("bk", [D], F32, kind="ExternalInput").ap()
    bv = nc.dram_tensor("bv", [D], F32, kind="ExternalInput").ap()
    out = nc.dram_tensor("out", [S, D], F32, kind="ExternalOutput").ap()

    with tile.TileContext(nc) as tc:
        _build(nc, tc, q, k, v, Wq, Wk, Wv, bq, bk, bv, out)
    nc.compile()
    return nc


def _build(nc, tc, q, k, v, Wq, Wk, Wv, bq, bk, bv, out):
    with ExitStack() as ctx:
        _build_inner(nc, tc, ctx, q, k, v, Wq, Wk, Wv, bq, bk, bv, out)


def _build_inner(nc, tc, ctx, q, k, v, Wq, Wk, Wv, bq, bk, bv, out):
    # ---- pools -------------------------------------------------------
    const = ctx.enter_context(tc.tile_pool(name="const", bufs=1))
    resid = ctx.enter_context(tc.tile_pool(name="resid", bufs=1))
    io = ctx.enter_context(tc.tile_pool(name="io", bufs=8))
    work = ctx.enter_context(tc.tile_pool(name="work", bufs=3))
    stats = ctx.enter_context(tc.tile_pool(name="stats", bufs=4))
    psT = ctx.enter_context(tc.tile_pool(name="psT", bufs=3, space="PSUM"))
    psS = ctx.enter_context(tc.tile_pool(name="psS", bufs=3, space="PSUM"))
    psO = ctx.enter_context(tc.tile_pool(name="psO", bufs=2, space="PSUM"))

    # ---- group DMA machinery -----------------------------------------
    # Every x input is streamed as ONE dma_start per 1MB 4-tile group
    # ([128, 4, 512]) instead of four tile DMAs: the in-order SP
    # sequencer pays one dispatch + at most one buffer-free wait per
    # group (measured v5: 66us of sequencer EVENT_SEMAPHORE waits +
    # 63 dispatches serialized the whole input stream to ~140GB/s).
    # Groups are issued ~2 stages before use, AFTER the previous
    # tenant's readers are emitted, so the ring never blocks long.
    k_t = k.rearrange("(t c p) d -> p t c d", p=P, c=4)
    v_t = v.rearrange("(t p) d -> p t d", p=P)
    q_t = q.rearrange("(c s p) d -> p c s d", p=P, c=NCH)
    grp = {}

    def issue_group(key, src_ap):
        g = io.tile([P, NT_D, D], F32R, tag="xg", name=f"g_{key[0]}{key[1]}",
                    bufs=2)
        nc.sync.dma_start(g[:], src_ap)
        grp[key] = g

    # ---- head DMA: Wk first (gates the W transposes that gate all of
    # kh), then bkb (needed by kh_M(0)), then two k groups.
    Wn_k = work.tile([P, NT_D, D], F32R, tag="wn", name="Wn_k", bufs=1)
    nc.sync.dma_start(Wn_k[:],
                      Wk.rearrange("(t p) d -> p t d", p=P).bitcast(F32R))
    bkb = const.tile([P, D], F32, name="bkb")
    nc.sync.dma_start(bkb[:], bk[None, :].to_broadcast((P, D)))
    issue_group(("k", 0), k_t[:, 0, :, :].bitcast(F32R))
    issue_group(("k", 1), k_t[:, 1, :, :].bitcast(F32R))

    # ---- PE warm-up: dummy matmuls fill the initial DMA wait and bring
    # the HAM clock to 2.4 GHz before real work arrives (values unused).
    warm = const.tile([P, D], BF16, name="warm")
    nc.gpsimd.memset(warm[:], 0.0)
    wps = psO.tile([P, D], F32, tag="o512", name="warm_ps")
    for _ in range(N_WARM):
        nc.tensor.matmul(wps[:], warm[:, :P], warm[:], start=True, stop=True)

    # ---- constants ---------------------------------------------------
    ident_f32 = const.tile([P, P], F32, name="ident_f32")
    make_identity(nc, ident_f32)
    ident_f32r_t = const.tile([P, P], F32R, name="ident_f32r_t")
    nc.vector.tensor_copy(ident_f32r_t[:], ident_f32[:])
    ident_f32r = ident_f32r_t[:]
    ident_bf16 = const.tile([P, P], BF16, name="ident_bf16")
    make_identity(nc, ident_bf16)

    # ---- residents ---------------------------------------------------
    qhT = resid.tile([P, NT_D, S], F32R, tag="qhT", name="qhT")      # [e, i]
    vh = resid.tile([P, NT_S, D], BF16, tag="vh", name="vh")         # [j, e]
    kr = resid.tile([P, NT_D, S], F32R, tag="kr", name="kr")         # [e', j]
    WTv = resid.tile([P, NT_S, P], BF16, tag="WTv", name="WTv")      # xbar'd
    # [d_part, t, e'] view of WTv: block t*4+dt holds Wv[t-block, dt-block]^T
    WTv_de = WTv[:].rearrange("p (t d) c -> p d t c", d=NT_D)
    WT = {}

    # late-issued DMA tiles/constants (filled by the stages below)
    bvb = const.tile([P, D], F32, name="bvb")
    bqT = const.tile([P, NT_D], F32, name="bqT")

    # ================= projection stage pipeline ======================
    # Each stage is (t_fn, m_fn).  Executed with one stage of lookahead:
    # T(0) T(1) M(0) T(2) M(1) ... so stage n's staging copies hide
    # under stage n+1's transposes plus stage n-1's matmuls.

    def wload_T(name, Wn_src, ets=None):
        def f():
            Wn = Wn_src()
            if name not in WT:
                WT[name] = resid.tile([P, NT_D, D], F32R, tag=f"WT_{name}",
                                      name=f"WT_{name}")
            WTt = WT[name]
            for et in (range(NT_D) if ets is None else ets):
                ps = psT.tile([P, NT_D, P], F32R, tag="ps128", name="wt_ps")
                for dt in range(NT_D):
                    nc.tensor.transpose(ps[:, dt, :],
                                        Wn[:, et, dt * P:(dt + 1) * P],
                                        ident_f32r)
                nc.scalar.copy(WTt[:, :, et * P:(et + 1) * P], ps[:])
        return f

    # ---- kh stages: transpose k tiles (PE, fp32r) with a stride-4
    # packed staging copy, then project with WT_k as the moving operand
    # straight into kr's reshape layout.
    kh_state = {}

    def kh_T(t2, post=None):
        def f():
            xTg = work.tile([P, NT_D, 512], F32R, tag="qTc", name="kTg",
                            bufs=2)
            xTg_p = xTg[:].rearrange("p d (a u) -> p d a u", a=4)
            kg = grp.pop(("k", t2))
            for c in range(4):
                ps = psT.tile([P, NT_D, P], F32R, tag="ps128", name="kt_ps")
                for dt in range(NT_D):
                    nc.tensor.transpose(ps[:, dt, :],
                                        kg[:, c, dt * P:(dt + 1) * P],
                                        ident_f32r)
                # packed write: column x of ps goes to (a=x%4, u=32c+x//4)
                nc.scalar.copy(xTg_p[:, :, :, 32 * c:32 * c + 32],
                               ps[:].rearrange("p d (u a) -> p d a u", a=4))
            kh_state[t2] = xTg
            if post is not None:
                post()
        return f

    def kh_M(t2):
        def f():
            xTg = kh_state.pop(t2)
            for a in range(4):
                pp = psS.tile([P, D], F32, tag="ps512", name="kh_ps")
                for dt in range(NT_D):
                    nc.tensor.matmul(pp[:], xTg[:, dt, a * P:(a + 1) * P],
                                     WT["k"][:, dt, :],
                                     start=(dt == 0), stop=(dt == NT_D - 1))
                nc.vector.tensor_tensor(kr[:, t2, a * 512:(a + 1) * 512],
                                        pp[:], bkb[:], op=ALU.add)
        return f

    # ---- v path: bf16 + DMA XBAR, no PE work in the T stages.
    # Casts are split between ScalarE and DVE per group (GpSimd takes
    # 2.1us per [128,512] cast - measured, 4x slower than DVE - and
    # single-handedly starved the PE in the v2 kernel).
    vh_state = {}
    wv_state = {}

    def wv_T():
        # cast Wv f32->bf16 in one 2048-wide DVE op, then XBAR it.
        # The bf16 buffer borrows the vTx tag (same 4KB/partition
        # footprint) so the v-group cast buffers never contend with it.
        def f():
            wbf = work.tile([P, NT_D, D], BF16, tag="vTx", name="Wv_bf",
                            bufs=2)
            nc.vector.tensor_copy(wbf[:], wv_state["wn"][:].bitcast(F32))
            nc.scalar.dma_start(WTv[:], wbf[:], transpose=True)
        return f

    def issue_wv_dma():
        wn = work.tile([P, NT_D, D], F32R, tag="wn", name="Wn_v", bufs=1)
        nc.sync.dma_start(
            wn[:], Wv.rearrange("(t p) d -> p t d", p=P).bitcast(F32R))
        nc.sync.dma_start(bvb[:], bv[None, :].to_broadcast((P, D)))
        wv_state["wn"] = wn

    def vh_T(g, post=None):
        def f():
            vbf = work.tile([P, NT_D, D], BF16, tag="vbf", name="vbf",
                            bufs=2)
            vg = grp.pop(("v", g))
            # one 1024-wide cast on ScalarE, one on DVE
            nc.scalar.copy(vbf[:, 0:2, :], vg[:, 0:2, :].bitcast(F32))
            nc.vector.tensor_copy(vbf[:, 2:4, :], vg[:, 2:4, :].bitcast(F32))
            vTx = work.tile([P, NT_S, P], BF16, tag="vTx", name="vTx",
                            bufs=2)
            nc.scalar.dma_start(vTx[:], vbf[:], transpose=True)
            vh_state[g] = vTx
            if post is not None:
                post()
        return f

    def vh_M(g):
        def f():
            vTx = vh_state.pop(g)
            for c in range(4):
                st = 4 * g + c
                pp = psS.tile([P, D], F32, tag="ps512", name="vh_ps")
                for dt in range(NT_D):
                    nc.tensor.matmul(pp[:], vTx[:, 4 * c + dt, :],
                                     WTv_de[:, dt, :, :],
                                     start=(dt == 0), stop=(dt == NT_D - 1))
                nc.vector.tensor_tensor(vh[:, st, :], pp[:], bvb[:],
                                        op=ALU.add)
        return f

    def issue_wq_dma():
        # (ACT-ring input DMAs corrupt data when mixed with the XBAR
        # transposes on that ring - measured rel err 180; keep ALL
        # plain input DMAs on the SP ring.)
        wn = work.tile([P, NT_D, D], F32R, tag="wn", name="Wn_q", bufs=1)
        nc.sync.dma_start(
            wn[:], Wq.rearrange("(t p) d -> p t d", p=P).bitcast(F32R))
        nc.sync.dma_start(bqT[:], bq.rearrange("(t p) -> p t", p=P))
        wq_state["wn"] = wn

    wq_state = {}

    # ---- qh stages: PE-transpose q (fp32r), project with WT_q
    # stationary into qhT [e_part, i]; bias rides DVE (ScalarE is busy
    # with the qTc staging copies in this phase).
    qh_state = {}

    def qh_T(ic, post=None):
        def f():
            qTc = work.tile([P, NT_D, 512], F32R, tag="qTc", name="qTc",
                            bufs=2)
            qg = grp.pop(("q", ic))
            for c in range(4):
                ps = psT.tile([P, NT_D, P], F32R, tag="ps128", name="qt_ps")
                for dt in range(NT_D):
                    nc.tensor.transpose(ps[:, dt, :],
                                        qg[:, c, dt * P:(dt + 1) * P],
                                        ident_f32r)
                nc.scalar.copy(qTc[:, :, c * P:(c + 1) * P], ps[:])
            qh_state[ic] = qTc
            if post is not None:
                post()
        return f

    def qh_M(ic):
        def f():
            qTc = qh_state.pop(ic)
            for et in range(NT_D):
                pp = psS.tile([P, 512], F32, tag="ps512", name="qh_ps")
                for dt in range(NT_D):
                    nc.tensor.matmul(pp[:],
                                     WT["q"][:, dt, et * P:(et + 1) * P],
                                     qTc[:, dt, :],
                                     start=(dt == 0), stop=(dt == NT_D - 1))
                nc.vector.tensor_scalar_add(
                    qhT[:, et, ic * 512:(ic + 1) * 512], pp[:],
                    bqT[:, et:et + 1])
        return f

    # ---- stage schedule (T-column, M-column) -------------------------
    # Wv/Wq DMAs are issued early from inside kh stages (the ring is
    # in-order; each lands ~2+ stages before its consumers).  The Wv
    # bf16 cast rides DVE in two chunks during kh; the Wq transposes
    # fill the PE hole left by vh_T(1) (which has no PE work).
    def seq(*fns):
        def f():
            for g in fns:
                g()
        return f

    # Wn lifetimes through the single "wn" buffer are DISJOINT: Wk
    # (head -> S0), Wq (issued S1, transposed S2/S3), Wv (issued S3
    # after Wq's last readers are emitted, cast S4).  An early-DMA'd-
    # late-consumed weight in this buffer blocks the next weight's DMA,
    # and the in-order SP ring then head-of-line-blocks every tile
    # behind it (measured: a ~30us stall of the whole input stream).
    # Every group DMA is issued as a `post` of the stage that emits the
    # previous buffer tenant's readers - both so the wait is satisfied
    # quickly and so the pool dependency tracking sees those readers.
    wq_src = lambda: wq_state["wn"]  # noqa: E731
    kg = lambda t2: (lambda: issue_group(("k", t2), k_t[:, t2, :, :].bitcast(F32R)))  # noqa: E731,E501
    vg = lambda g: (lambda: issue_group(("v", g), v_t[:, 4 * g:4 * g + 4, :].bitcast(F32R)))  # noqa: E731,E501
    qg = lambda ic: (lambda: issue_group(("q", ic), q_t[:, ic, :, :].bitcast(F32R)))  # noqa: E731,E501
    stages = [
        (wload_T("k", lambda: Wn_k), None),
        (kh_T(0, post=seq(kg(2), issue_wq_dma)), None),
        (seq(kh_T(1, post=kg(3)), wload_T("q", wq_src, ets=(0, 1))), kh_M(0)),
        (seq(kh_T(2, post=vg(0)), wload_T("q", wq_src, ets=(2, 3)),
             issue_wv_dma), kh_M(1)),
        (seq(kh_T(3, post=vg(1)), wv_T()), kh_M(2)),
        (vh_T(0, post=vg(2)), kh_M(3)),
        (vh_T(1, post=vg(3)), vh_M(0)),
        (vh_T(2, post=qg(0)), vh_M(1)),
        (vh_T(3, post=qg(1)), vh_M(2)),
        (qh_T(0, post=qg(2)), vh_M(3)),
        (qh_T(1, post=qg(3)), qh_M(0)),
        (qh_T(2), qh_M(1)),
        (qh_T(3), qh_M(2)),
        (None, qh_M(3)),
    ]
    for t_fn, m_fn in stages:
        if t_fn is not None:
            t_fn()
        if m_fn is not None:
            m_fn()

    # ---- attention per 128-row i-block -------------------------------
    def scores_softmax(ib):
        mx = stats.tile([P, NCH], F32, tag="mx", name="mx")
        p_bf = work.tile([P, S], BF16, tag="p", name="p_bf", bufs=2)

        s_sb = work.tile([P, NCH, 512], F32, tag="s_sb", name="s_sb", bufs=2)
        for jc in range(NCH):
            sp = psS.tile([P, 512], F32, tag="ps512", name="s_ps")
            for et in range(NT_D):
                nc.tensor.matmul(sp[:],
                                 qhT[:, et, ib * P:(ib + 1) * P],
                                 kr[:, et, jc * 512:(jc + 1) * 512],
                                 start=(et == 0), stop=(et == NT_D - 1))
            # fused PSUM->SBUF stage + row-max pair: the copy frees the
            # PSUM bank fast (it is the bank's only reader), the max runs
            # on the SBUF copy.  (tensor_tensor_reduce from PSUM wedges
            # the HW - keep the plain copy + max pair.)
            nc.vector.tensor_copy(s_sb[:, jc, :], sp[:])
            nc.vector.reduce_max(mx[:, jc:jc + 1], s_sb[:, jc, :],
                                 axis=AX.X)

        gmx = stats.tile([P, 1], F32, tag="gmx", name="gmx")
        ngmx = stats.tile([P, 1], F32, tag="ngmx", name="ngmx")
        den = stats.tile([P, 1], F32, tag="den", name="den")
        rs = stats.tile([P, 1], F32, tag="rs", name="rs")
        nc.vector.reduce_max(gmx[:], mx[:], axis=AX.X)
        nc.vector.tensor_scalar_mul(ngmx[:], gmx[:], -SQRT_D)

        # one exp over the whole 2048-wide row; the accumulator read IS
        # the softmax denominator.
        nc.scalar.activation(p_bf[:], s_sb[:], AF.Exp, bias=ngmx[:, 0:1],
                             scale=SQRT_D, accum_out=den[:])
        nc.vector.reciprocal(rs[:], den[:])

        pT = work.tile([P, NT_S, P], BF16, tag="pT", name="pT", bufs=4)
        # Last two blocks: no further scores hide the XBAR dispatch+stream
        # chain at pipeline drain, but the PE is idling there - PE
        # transposes are faster for exactly those blocks.
        if XBAR_PROBS and ib < NT_S - 2:
            nc.scalar.dma_start(pT[:], p_bf[:], transpose=True)
        else:
            for a in range(NCH):
                ps = psT.tile([P, NT_D, P], BF16, tag="ps128", name="pt_ps")
                for b2 in range(NT_D):
                    jt = a * NT_D + b2
                    nc.tensor.transpose(ps[:, b2, :],
                                        p_bf[:, jt * P:(jt + 1) * P],
                                        ident_bf16[:])
                nc.vector.tensor_copy(pT[:, a * NT_D:(a + 1) * NT_D, :], ps[:])
        return pT, rs

    def attend_out(ib, pT, rs):
        op = psO.tile([P, D], F32, tag="o512", name="o_ps")
        for jt in range(NT_S):
            nc.tensor.matmul(op[:], pT[:, jt, :], vh[:, jt, :],
                             start=(jt == 0), stop=(jt == NT_S - 1))
        # 1/den scale on DVE (keeps ScalarE free for the exp/XBAR chain;
        # a fused scale+bias scalar_tensor_tensor here costs 900ns vs
        # 430ns - 2-input DVE ops are 2x - and DVE is the block's
        # critical path: measured +20us on the attention phase)
        o_sb = work.tile([P, D], F32, tag="o_sb", name="o_sb", bufs=2)
        nc.vector.tensor_scalar_mul(o_sb[:], op[:], rs[:, 0:1])
        nc.sync.dma_start(out.rearrange("(t p) e -> p t e", p=P)[:, ib, :],
                          o_sb[:])

    # 4-deep: the scores->exp->XBAR->a@vh chain spans ~2 block periods;
    # three blocks of scores/softmax ahead of each a@vh absorbs the
    # chain's jitter (exp/dispatch variance) entirely.
    pend = [scores_softmax(0), scores_softmax(1), scores_softmax(2)]
    for ib in range(NT_S):
        if ib + 3 < NT_S:
            pend.append(scores_softmax(ib + 3))
        attend_out(ib, *pend.pop(0))


def _ensure_axon_hooks_module():
    """antenv.axon_hooks is missing on this image; provide it (with the real
    ctypes NTFF hook when available) so run_bass_kernel_spmd(trace=True)
    degrades gracefully instead of raising ImportError."""
    import sys
    import types
    try:
        import antenv
        import antenv.axon_hooks  # noqa: F401
        return
    except ImportError:
        pass
    try:
        mod = types.ModuleType("antenv.axon_hooks")
        state = {"hook": None}
        mod.set_axon_ntff_profile_hook = lambda h: state.__setitem__("hook", h)
        mod.get_axon_ntff_profile_hook = lambda: state["hook"]
        sys.modules["antenv.axon_hooks"] = mod
        antenv.axon_hooks = mod
        try:
            if "/root/.axon_site" not in sys.path:
                sys.path.insert(0, "/root/.axon_site")
            from trn_agent_boot.trn_boot import _ntff_profile_via_ctypes

            mod.set_axon_ntff_profile_hook(
                _ntff_profile_via_ctypes("/opt/axon/libaxon_pjrt.so")
            )
        except Exception:
            pass
    except Exception:
        pass


_ensure_axon_hooks_module()

_NC_CACHE = None


def _get_nc():
    global _NC_CACHE
    if _NC_CACHE is None:
        _NC_CACHE = build_nc()
    return _NC_CACHE


def kernel(q, k, v, Wq, bq, Wk, bk, Wv, bv):
    nc = _get_nc()
    in_maps = []
    for b in range(B):
        in_maps.append({
            "q": np.ascontiguousarray(q[b], dtype=np.float32),
            "k": np.ascontiguousarray(k[b], dtype=np.float32),
            "v": np.ascontiguousarray(v[b], dtype=np.float32),
            "Wq": np.ascontiguousarray(Wq, dtype=np.float32),
            "Wk": np.ascontiguousarray(Wk, dtype=np.float32),
            "Wv": np.ascontiguousarray(Wv, dtype=np.float32),
            "bq": np.ascontiguousarray(bq, dtype=np.float32),
            "bk": np.ascontiguousarray(bk, dtype=np.float32),
            "bv": np.ascontiguousarray(bv, dtype=np.float32),
        })
    res = run_bass_kernel_spmd(nc, in_maps, core_ids=list(range(B)))
    return np.stack([res.results[b]["out"] for b in range(B)], axis=0)


# revision 32
# speedup vs baseline: 1.0065x; 1.0065x over previous
"""Trainium2 Bass kernel for nn_AttentionHead (B=8, S=2048, DK=512).

Reference semantics (faithful to the source module, bugs included):
    qh = q @ Wq.T + bq            # [B, S, D]
    kh = k @ Wk.T + bk
    vh = v @ Wv.T + bv
    kr = kh.reshape(B, D, S)      # row-major REINTERPRET, not a transpose
    s  = (qh @ kr) * sqrt(D)      # source bug: multiplies by sqrt(D)
    a  = softmax(s, axis=2)
    out = a @ vh                  # [B, S, dk]

Sharding: data-parallel over batch - one batch element per NeuronCore,
8 cores, no collectives. Each core runs the same NEFF with its own shard.

Per-core dataflow (v2 - stage-pipelined projections):
  - q,k and Wq,Wk are PE-transposed in fp32r (scores need tf32 accuracy:
    the kept source bug multiplies scores by sqrt(D), making the softmax
    near-one-hot and argmax-sensitive; bf16 q/k fails the error budget).
  - The v path is bf16 end-to-end: v tiles are cast f32->bf16 on GpSimd
    (an otherwise idle engine) and transposed by the DMA XBAR in one
    [128,2048]->[128,16,128] shot per 4-tile group; Wv likewise (cast +
    one XBAR shot) - no PE transposes and no PSUM->SBUF staging copies
    on the whole v path.  vh only feeds the bf16 a@vh matmul, and out
    is a near-one-hot selection of vh rows, so bf16 projection inputs
    cost ~0.2% extra error against a 2e-2 budget.
  - All projection loops are software-pipelined one stage deep: the
    transposes (+DMA) of stage n+1 are emitted BEFORE the matmuls of
    stage n, so the PE-transpose -> ScalarE-staging-copy -> matmul
    chain's copy latency hides under the previous stage's matmuls.
    (The v1 kernel serialized T(n)->copy(n)->M(n) per iteration and
    lost ~300-500ns of PE time per iteration waiting on ScalarE.)
  - DMA head order: Wk tiles first (they gate the Wk transposes that
    gate everything), then bkb, then two k tile-groups; Wv/bvb are
    issued from inside kh stage 3, Wq/bqT from inside vh stage 2, so
    each weight lands ~2 stages before its consumers without delaying
    the k/v tile stream.  (v1 put 1MB of k tiles ahead of Wk and the
    first real PE work started at 14.6us; now ~6us.)
  - kr = kh.reshape(D, S) is produced with no data movement: the kh
    projection picks its stationary columns with a stride-4 AP packing
    done by the PSUM->SBUF staging copy (ScalarE), which permutes the
    PSUM partition order into kr's layout; the bias-add writes straight
    into the resident kr tile.
  - scores: s[i_part, j] accumulated in PSUM with fp32r (tf32) matmuls.
  - softmax: each 512-chunk PSUM is copied to SBUF (DVE) immediately so
    the bank frees fast; ONE ScalarE exp covers the whole 2048-wide row
    and its accumulator read IS the softmax denominator.
  - probs are transposed by the DMA XBAR (one [128,2048]->[128,16,128]
    shot per i-block) on the ACT HWDGE ring (the SP ring strands the
    transpose's tail descriptors until the next SP dispatch -> ~50%
    stale data).  The attention loop is software-pipelined 4 deep; the
    last two blocks transpose probs on the (then-idle) PE instead.
  - qh bias rides a DVE tensor_scalar_add (ScalarE in the qh phase is
    loaded with the qTc staging copies; DVE is idle there).
  - dummy bf16 matmuls at kernel start fill the initial DMA wait and
    warm the PE clock gate to 2.4 GHz before real work arrives.

Known dead ends (measured, do not retry): v-path PER-TILE XBAR (16
dispatches x ~1.1us fixed cost on ScalarE dwarf the PE saving - batch
4 tiles per shot instead); io bufs < 8; tensor_tensor_reduce with a
PSUM input (wedges the device); per-chunk XBAR splits; PE-transposing
block 13's probs; fp8/DoubleRow (accuracy: near-one-hot softmax is
argmax-sensitive, fp8 value paths exceed the 2e-2 budget); fp32 XBAR
transpose (HW caps 4-byte transposes at 64 output partitions).
"""

from contextlib import ExitStack

import numpy as np

import concourse.bacc as bacc
import concourse.mybir as mybir
import concourse.tile as tile
from concourse.bass_utils import run_bass_kernel_spmd
from concourse.masks import make_identity

AF = mybir.ActivationFunctionType
ALU = mybir.AluOpType
AX = mybir.AxisListType
F32 = mybir.dt.float32
F32R = mybir.dt.float32r
BF16 = mybir.dt.bfloat16

B, S, D = 8, 2048, 512
P = 128
NT_S = S // P          # 16 s-tiles (also j-tiles / i-blocks)
NT_D = D // P          # 4 d-tiles (also e-tiles)
NCH = S // 512         # 4 512-wide chunks of the sequence dim
SQRT_D = float(np.sqrt(np.float32(D)))

N_WARM = 12            # PE warm-up matmuls (fill DMA head, warm clock)
XBAR_PROBS = True      # probs transpose via DMA XBAR instead of PE


def build_nc():
    nc = bacc.Bacc("TRN2", target_bir_lowering=False, debug=False,
                   enable_asserts=False, num_devices=B)

    q = nc.dram_tensor("q", [S, D], F32, kind="ExternalInput").ap()
    k = nc.dram_tensor("k", [S, D], F32, kind="ExternalInput").ap()
    v = nc.dram_tensor("v", [S, D], F32, kind="ExternalInput").ap()
    Wq = nc.dram_tensor("Wq", [D, D], F32, kind="ExternalInput").ap()
    Wk = nc.dram_tensor("Wk", [D, D], F32, kind="ExternalInput").ap()
    Wv = nc.dram_tensor("Wv", [D, D], F32, kind="ExternalInput").ap()
    bq = nc.dram_tensor("bq", [D], F32, kind="ExternalInput").ap()
    bk = nc.dram_tensor("bk", [D], F32, kind="ExternalInput").ap()
    bv = nc.dram_tensor("bv", [D], F32, kind="ExternalInput").ap()
    out = nc.dram_tensor("out", [S, D], F32, kind="ExternalOutput").ap()

    with tile.TileContext(nc) as tc:
        _build(nc, tc, q, k, v, Wq, Wk, Wv, bq, bk, bv, out)
    nc.compile()
    return nc


def _build(nc, tc, q, k, v, Wq, Wk, Wv, bq, bk, bv, out):
    with ExitStack() as ctx:
        _build_inner(nc, tc, ctx, q, k, v, Wq, Wk, Wv, bq, bk, bv, out)


def _build_inner(nc, tc, ctx, q, k, v, Wq, Wk, Wv, bq, bk, bv, out):
    # ---- pools -------------------------------------------------------
    const = ctx.enter_context(tc.tile_pool(name="const", bufs=1))
    resid = ctx.enter_context(tc.tile_pool(name="resid", bufs=1))
    io = ctx.enter_context(tc.tile_pool(name="io", bufs=8))
    work = ctx.enter_context(tc.tile_pool(name="work", bufs=3))
    stats = ctx.enter_context(tc.tile_pool(name="stats", bufs=4))
    psT = ctx.enter_context(tc.tile_pool(name="psT", bufs=3, space="PSUM"))
    psS = ctx.enter_context(tc.tile_pool(name="psS", bufs=3, space="PSUM"))
    psO = ctx.enter_context(tc.tile_pool(name="psO", bufs=2, space="PSUM"))

    # ---- group DMA machinery -----------------------------------------
    # Every x input is streamed as ONE dma_start per 1MB 4-tile group
    # ([128, 4, 512]) instead of four tile DMAs: the in-order SP
    # sequencer pays one dispatch + at most one buffer-free wait per
    # group (measured v5: 66us of sequencer EVENT_SEMAPHORE waits +
    # 63 dispatches serialized the whole input stream to ~140GB/s).
    # Groups are issued ~2 stages before use, AFTER the previous
    # tenant's readers are emitted, so the ring never blocks long.
    k_t = k.rearrange("(t c p) d -> p t c d", p=P, c=4)
    v_t = v.rearrange("(t p) d -> p t d", p=P)
    q_t = q.rearrange("(c s p) d -> p c s d", p=P, c=NCH)
    grp = {}

    def issue_group(key, src_ap):
        # 4 sub-DMAs per group: a single 1MB DMA instruction is capped
        # at ~80GB/s by the per-instruction descriptor issue rate, but
        # concurrent instructions scale; the 4 sub-DMAs share ONE
        # buffer-free wait (only the first pays it), so the sequencer
        # stays cheap.
        g = io.tile([P, NT_D, D], F32R, tag="xg", name=f"g_{key[0]}{key[1]}",
                    bufs=2)
        for c in range(NT_D):
            nc.sync.dma_start(g[:, c, :], src_ap[:, c, :])
        grp[key] = g

    # ---- head DMA: Wk first (gates the W transposes that gate all of
    # kh), then bkb (needed by kh_M(0)), then two k groups.
    Wn_k = work.tile([P, NT_D, D], F32R, tag="wn", name="Wn_k", bufs=1)
    Wk_r = Wk.rearrange("(t p) d -> p t d", p=P).bitcast(F32R)
    for et in range(NT_D):
        nc.sync.dma_start(Wn_k[:, et, :], Wk_r[:, et, :])
    bkb = const.tile([P, D], F32, name="bkb")
    nc.sync.dma_start(bkb[:], bk[None, :].to_broadcast((P, D)))
    issue_group(("k", 0), k_t[:, 0, :, :].bitcast(F32R))
    issue_group(("k", 1), k_t[:, 1, :, :].bitcast(F32R))

    # ---- PE warm-up: dummy matmuls fill the initial DMA wait and bring
    # the HAM clock to 2.4 GHz before real work arrives (values unused).
    warm = const.tile([P, D], BF16, name="warm")
    nc.gpsimd.memset(warm[:], 0.0)
    wps = psO.tile([P, D], F32, tag="o512", name="warm_ps")
    for _ in range(N_WARM):
        nc.tensor.matmul(wps[:], warm[:, :P], warm[:], start=True, stop=True)

    # ---- constants ---------------------------------------------------
    ident_f32 = const.tile([P, P], F32, name="ident_f32")
    make_identity(nc, ident_f32)
    ident_f32r_t = const.tile([P, P], F32R, name="ident_f32r_t")
    nc.vector.tensor_copy(ident_f32r_t[:], ident_f32[:])
    ident_f32r = ident_f32r_t[:]
    ident_bf16 = const.tile([P, P], BF16, name="ident_bf16")
    make_identity(nc, ident_bf16)

    # ---- residents ---------------------------------------------------
    qhT = resid.tile([P, NT_D, S], F32R, tag="qhT", name="qhT")      # [e, i]
    vh = resid.tile([P, NT_S, D], BF16, tag="vh", name="vh")         # [j, e]
    kr = resid.tile([P, NT_D, S], F32R, tag="kr", name="kr")         # [e', j]
    WTv = resid.tile([P, NT_S, P], BF16, tag="WTv", name="WTv")      # xbar'd
    # [d_part, t, e'] view of WTv: block t*4+dt holds Wv[t-block, dt-block]^T
    WTv_de = WTv[:].rearrange("p (t d) c -> p d t c", d=NT_D)
    WT = {}

    # late-issued DMA tiles/constants (filled by the stages below)
    bvb = const.tile([P, D], F32, name="bvb")
    bqT = const.tile([P, NT_D], F32, name="bqT")

    # ================= projection stage pipeline ======================
    # Each stage is (t_fn, m_fn).  Executed with one stage of lookahead:
    # T(0) T(1) M(0) T(2) M(1) ... so stage n's staging copies hide
    # under stage n+1's transposes plus stage n-1's matmuls.

    def wload_T(name, Wn_src, ets=None):
        def f():
            Wn = Wn_src()
            if name not in WT:
                WT[name] = resid.tile([P, NT_D, D], F32R, tag=f"WT_{name}",
                                      name=f"WT_{name}")
            WTt = WT[name]
            for et in (range(NT_D) if ets is None else ets):
                ps = psT.tile([P, NT_D, P], F32R, tag="ps128", name="wt_ps")
                for dt in range(NT_D):
                    nc.tensor.transpose(ps[:, dt, :],
                                        Wn[:, et, dt * P:(dt + 1) * P],
                                        ident_f32r)
                nc.scalar.copy(WTt[:, :, et * P:(et + 1) * P], ps[:])
        return f

    # ---- kh stages: transpose k tiles (PE, fp32r) with a stride-4
    # packed staging copy, then project with WT_k as the moving operand
    # straight into kr's reshape layout.
    kh_state = {}

    def kh_T(t2, post=None):
        def f():
            xTg = work.tile([P, NT_D, 512], F32R, tag="qTc", name="kTg",
                            bufs=2)
            xTg_p = xTg[:].rearrange("p d (a u) -> p d a u", a=4)
            kg = grp.pop(("k", t2))
            for c in range(4):
                ps = psT.tile([P, NT_D, P], F32R, tag="ps128", name="kt_ps")
                for dt in range(NT_D):
                    nc.tensor.transpose(ps[:, dt, :],
                                        kg[:, c, dt * P:(dt + 1) * P],
                                        ident_f32r)
                # packed write: column x of ps goes to (a=x%4, u=32c+x//4)
                nc.scalar.copy(xTg_p[:, :, :, 32 * c:32 * c + 32],
                               ps[:].rearrange("p d (u a) -> p d a u", a=4))
            kh_state[t2] = xTg
            if post is not None:
                post()
        return f

    def kh_M(t2):
        def f():
            xTg = kh_state.pop(t2)
            for a in range(4):
                pp = psS.tile([P, D], F32, tag="ps512", name="kh_ps")
                for dt in range(NT_D):
                    nc.tensor.matmul(pp[:], xTg[:, dt, a * P:(a + 1) * P],
                                     WT["k"][:, dt, :],
                                     start=(dt == 0), stop=(dt == NT_D - 1))
                nc.vector.tensor_tensor(kr[:, t2, a * 512:(a + 1) * 512],
                                        pp[:], bkb[:], op=ALU.add)
        return f

    # ---- v path: bf16 + DMA XBAR, no PE work in the T stages.
    # Casts are split between ScalarE and DVE per group (GpSimd takes
    # 2.1us per [128,512] cast - measured, 4x slower than DVE - and
    # single-handedly starved the PE in the v2 kernel).
    vh_state = {}
    wv_state = {}

    def wv_T():
        # cast Wv f32->bf16 in one 2048-wide DVE op, then XBAR it.
        # The bf16 buffer borrows the vTx tag (same 4KB/partition
        # footprint) so the v-group cast buffers never contend with it.
        def f():
            wbf = work.tile([P, NT_D, D], BF16, tag="vTx", name="Wv_bf",
                            bufs=2)
            nc.vector.tensor_copy(wbf[:], wv_state["wn"][:].bitcast(F32))
            nc.scalar.dma_start(WTv[:], wbf[:], transpose=True)
        return f

    def issue_wv_dma():
        wn = work.tile([P, NT_D, D], F32R, tag="wn", name="Wn_v", bufs=1)
        Wv_r = Wv.rearrange("(t p) d -> p t d", p=P).bitcast(F32R)
        for et in range(NT_D):
            nc.sync.dma_start(wn[:, et, :], Wv_r[:, et, :])
        nc.sync.dma_start(bvb[:], bv[None, :].to_broadcast((P, D)))
        wv_state["wn"] = wn

    def vh_T(g, post=None):
        def f():
            vbf = work.tile([P, NT_D, D], BF16, tag="vbf", name="vbf",
                            bufs=2)
            vg = grp.pop(("v", g))
            # one 1024-wide cast on ScalarE, one on DVE
            nc.scalar.copy(vbf[:, 0:2, :], vg[:, 0:2, :].bitcast(F32))
            nc.vector.tensor_copy(vbf[:, 2:4, :], vg[:, 2:4, :].bitcast(F32))
            vTx = work.tile([P, NT_S, P], BF16, tag="vTx", name="vTx",
                            bufs=2)
            nc.scalar.dma_start(vTx[:], vbf[:], transpose=True)
            vh_state[g] = vTx
            if post is not None:
                post()
        return f

    def vh_M(g):
        def f():
            vTx = vh_state.pop(g)
            for c in range(4):
                st = 4 * g + c
                pp = psS.tile([P, D], F32, tag="ps512", name="vh_ps")
                for dt in range(NT_D):
                    nc.tensor.matmul(pp[:], vTx[:, 4 * c + dt, :],
                                     WTv_de[:, dt, :, :],
                                     start=(dt == 0), stop=(dt == NT_D - 1))
                nc.vector.tensor_tensor(vh[:, st, :], pp[:], bvb[:],
                                        op=ALU.add)
        return f

    def issue_wq_dma():
        # (ACT-ring input DMAs corrupt data when mixed with the XBAR
        # transposes on that ring - measured rel err 180; keep ALL
        # plain input DMAs on the SP ring.)
        wn = work.tile([P, NT_D, D], F32R, tag="wn", name="Wn_q", bufs=1)
        Wq_r = Wq.rearrange("(t p) d -> p t d", p=P).bitcast(F32R)
        for et in range(NT_D):
            nc.sync.dma_start(wn[:, et, :], Wq_r[:, et, :])
        nc.sync.dma_start(bqT[:], bq.rearrange("(t p) -> p t", p=P))
        wq_state["wn"] = wn

    wq_state = {}

    # ---- qh stages: PE-transpose q (fp32r), project with WT_q
    # stationary into qhT [e_part, i]; bias rides DVE (ScalarE is busy
    # with the qTc staging copies in this phase).
    qh_state = {}

    def qh_T(ic, post=None):
        def f():
            qTc = work.tile([P, NT_D, 512], F32R, tag="qTc", name="qTc",
                            bufs=2)
            qg = grp.pop(("q", ic))
            for c in range(4):
                ps = psT.tile([P, NT_D, P], F32R, tag="ps128", name="qt_ps")
                for dt in range(NT_D):
                    nc.tensor.transpose(ps[:, dt, :],
                                        qg[:, c, dt * P:(dt + 1) * P],
                                        ident_f32r)
                nc.scalar.copy(qTc[:, :, c * P:(c + 1) * P], ps[:])
            qh_state[ic] = qTc
            if post is not None:
                post()
        return f

    def qh_M(ic):
        def f():
            qTc = qh_state.pop(ic)
            for et in range(NT_D):
                pp = psS.tile([P, 512], F32, tag="ps512", name="qh_ps")
                for dt in range(NT_D):
                    nc.tensor.matmul(pp[:],
                                     WT["q"][:, dt, et * P:(et + 1) * P],
                                     qTc[:, dt, :],
                                     start=(dt == 0), stop=(dt == NT_D - 1))
                nc.vector.tensor_scalar_add(
                    qhT[:, et, ic * 512:(ic + 1) * 512], pp[:],
                    bqT[:, et:et + 1])
        return f

    # ---- stage schedule (T-column, M-column) -------------------------
    # Wv/Wq DMAs are issued early from inside kh stages (the ring is
    # in-order; each lands ~2+ stages before its consumers).  The Wv
    # bf16 cast rides DVE in two chunks during kh; the Wq transposes
    # fill the PE hole left by vh_T(1) (which has no PE work).
    def seq(*fns):
        def f():
            for g in fns:
                g()
        return f

    # Wn lifetimes through the single "wn" buffer are DISJOINT: Wk
    # (head -> S0), Wq (issued S1, transposed S2/S3), Wv (issued S3
    # after Wq's last readers are emitted, cast S4).  An early-DMA'd-
    # late-consumed weight in this buffer blocks the next weight's DMA,
    # and the in-order SP ring then head-of-line-blocks every tile
    # behind it (measured: a ~30us stall of the whole input stream).
    # Every group DMA is issued as a `post` of the stage that emits the
    # previous buffer tenant's readers - both so the wait is satisfied
    # quickly and so the pool dependency tracking sees those readers.
    wq_src = lambda: wq_state["wn"]  # noqa: E731
    kg = lambda t2: (lambda: issue_group(("k", t2), k_t[:, t2, :, :].bitcast(F32R)))  # noqa: E731,E501
    vg = lambda g: (lambda: issue_group(("v", g), v_t[:, 4 * g:4 * g + 4, :].bitcast(F32R)))  # noqa: E731,E501
    qg = lambda ic: (lambda: issue_group(("q", ic), q_t[:, ic, :, :].bitcast(F32R)))  # noqa: E731,E501
    stages = [
        (wload_T("k", lambda: Wn_k), None),
        (kh_T(0, post=seq(kg(2), issue_wq_dma)), None),
        (seq(kh_T(1, post=kg(3)), wload_T("q", wq_src, ets=(0, 1))), kh_M(0)),
        (seq(kh_T(2, post=vg(0)), wload_T("q", wq_src, ets=(2, 3)),
             issue_wv_dma), kh_M(1)),
        (seq(kh_T(3, post=vg(1)), wv_T()), kh_M(2)),
        (vh_T(0, post=vg(2)), kh_M(3)),
        (vh_T(1, post=vg(3)), vh_M(0)),
        (vh_T(2, post=qg(0)), vh_M(1)),
        (vh_T(3, post=qg(1)), vh_M(2)),
        (qh_T(0, post=qg(2)), vh_M(3)),
        (qh_T(1, post=qg(3)), qh_M(0)),
        (qh_T(2), qh_M(1)),
        (qh_T(3), qh_M(2)),
        (None, qh_M(3)),
    ]
    for t_fn, m_fn in stages:
        if t_fn is not None:
            t_fn()
        if m_fn is not None:
            m_fn()

    # ---- attention per 128-row i-block -------------------------------
    def scores_softmax(ib):
        mx = stats.tile([P, NCH], F32, tag="mx", name="mx")
        p_bf = work.tile([P, S], BF16, tag="p", name="p_bf", bufs=2)

        s_sb = work.tile([P, NCH, 512], F32, tag="s_sb", name="s_sb", bufs=2)
        for jc in range(NCH):
            sp = psS.tile([P, 512], F32, tag="ps512", name="s_ps")
            for et in range(NT_D):
                nc.tensor.matmul(sp[:],
                                 qhT[:, et, ib * P:(ib + 1) * P],
                                 kr[:, et, jc * 512:(jc + 1) * 512],
                                 start=(et == 0), stop=(et == NT_D - 1))
            # fused PSUM->SBUF stage + row-max pair: the copy frees the
            # PSUM bank fast (it is the bank's only reader), the max runs
            # on the SBUF copy.  (tensor_tensor_reduce from PSUM wedges
            # the HW - keep the plain copy + max pair.)
            nc.vector.tensor_copy(s_sb[:, jc, :], sp[:])
            nc.vector.reduce_max(mx[:, jc:jc + 1], s_sb[:, jc, :],
                                 axis=AX.X)

        gmx = stats.tile([P, 1], F32, tag="gmx", name="gmx")
        ngmx = stats.tile([P, 1], F32, tag="ngmx", name="ngmx")
        den = stats.tile([P, 1], F32, tag="den", name="den")
        rs = stats.tile([P, 1], F32, tag="rs", name="rs")
        nc.vector.reduce_max(gmx[:], mx[:], axis=AX.X)
        nc.vector.tensor_scalar_mul(ngmx[:], gmx[:], -SQRT_D)

        # one exp over the whole 2048-wide row; the accumulator read IS
        # the softmax denominator.
        nc.scalar.activation(p_bf[:], s_sb[:], AF.Exp, bias=ngmx[:, 0:1],
                             scale=SQRT_D, accum_out=den[:])
        nc.vector.reciprocal(rs[:], den[:])

        pT = work.tile([P, NT_S, P], BF16, tag="pT", name="pT", bufs=4)
        # Last two blocks: no further scores hide the XBAR dispatch+stream
        # chain at pipeline drain, but the PE is idling there - PE
        # transposes are faster for exactly those blocks.
        if XBAR_PROBS and ib < NT_S - 2:
            nc.scalar.dma_start(pT[:], p_bf[:], transpose=True)
        else:
            for a in range(NCH):
                ps = psT.tile([P, NT_D, P], BF16, tag="ps128", name="pt_ps")
                for b2 in range(NT_D):
                    jt = a * NT_D + b2
                    nc.tensor.transpose(ps[:, b2, :],
                                        p_bf[:, jt * P:(jt + 1) * P],
                                        ident_bf16[:])
                nc.vector.tensor_copy(pT[:, a * NT_D:(a + 1) * NT_D, :], ps[:])
        return pT, rs

    def attend_out(ib, pT, rs):
        op = psO.tile([P, D], F32, tag="o512", name="o_ps")
        for jt in range(NT_S):
            nc.tensor.matmul(op[:], pT[:, jt, :], vh[:, jt, :],
                             start=(jt == 0), stop=(jt == NT_S - 1))
        # 1/den scale on DVE (keeps ScalarE free for the exp/XBAR chain;
        # a fused scale+bias scalar_tensor_tensor here costs 900ns vs
        # 430ns - 2-input DVE ops are 2x - and DVE is the block's
        # critical path: measured +20us on the attention phase)
        o_sb = work.tile([P, D], F32, tag="o_sb", name="o_sb", bufs=2)
        nc.vector.tensor_scalar_mul(o_sb[:], op[:], rs[:, 0:1])
        nc.sync.dma_start(out.rearrange("(t p) e -> p t e", p=P)[:, ib, :],
                          o_sb[:])

    # 4-deep: the scores->exp->XBAR->a@vh chain spans ~2 block periods;
    # three blocks of scores/softmax ahead of each a@vh absorbs the
    # chain's jitter (exp/dispatch variance) entirely.
    pend = [scores_softmax(0), scores_softmax(1), scores_softmax(2)]
    for ib in range(NT_S):
        if ib + 3 < NT_S:
            pend.append(scores_softmax(ib + 3))
        attend_out(ib, *pend.pop(0))


def _ensure_axon_hooks_module():
    """antenv.axon_hooks is missing on this image; provide it (with the real
    ctypes NTFF hook when available) so run_bass_kernel_spmd(trace=True)
    degrades gracefully instead of raising ImportError."""
    import sys
    import types
    try:
        import antenv
        import antenv.axon_hooks  # noqa: F401
        return
    except ImportError:
        pass
    try:
        mod = types.ModuleType("antenv.axon_hooks")
        state = {"hook": None}
        mod.set_axon_ntff_profile_hook = lambda h: state.__setitem__("hook", h)
        mod.get_axon_ntff_profile_hook = lambda: state["hook"]
        sys.modules["antenv.axon_hooks"] = mod
        antenv.axon_hooks = mod
        try:
            if "/root/.axon_site" not in sys.path:
                sys.path.insert(0, "/root/.axon_site")
            from trn_agent_boot.trn_boot import _ntff_profile_via_ctypes

            mod.set_axon_ntff_profile_hook(
                _ntff_profile_via_ctypes("/opt/axon/libaxon_pjrt.so")
            )
        except Exception:
            pass
    except Exception:
        pass


_ensure_axon_hooks_module()

_NC_CACHE = None


def _get_nc():
    global _NC_CACHE
    if _NC_CACHE is None:
        _NC_CACHE = build_nc()
    return _NC_CACHE


def kernel(q, k, v, Wq, bq, Wk, bk, Wv, bv):
    nc = _get_nc()
    in_maps = []
    for b in range(B):
        in_maps.append({
            "q": np.ascontiguousarray(q[b], dtype=np.float32),
            "k": np.ascontiguousarray(k[b], dtype=np.float32),
            "v": np.ascontiguousarray(v[b], dtype=np.float32),
            "Wq": np.ascontiguousarray(Wq, dtype=np.float32),
            "Wk": np.ascontiguousarray(Wk, dtype=np.float32),
            "Wv": np.ascontiguousarray(Wv, dtype=np.float32),
            "bq": np.ascontiguousarray(bq, dtype=np.float32),
            "bk": np.ascontiguousarray(bk, dtype=np.float32),
            "bv": np.ascontiguousarray(bv, dtype=np.float32),
        })
    res = run_bass_kernel_spmd(nc, in_maps, core_ids=list(range(B)))
    return np.stack([res.results[b]["out"] for b in range(B)], axis=0)


# revision 35
# speedup vs baseline: 1.0594x; 1.0525x over previous
"""Trainium2 Bass kernel for nn_AttentionHead (B=8, S=2048, DK=512).

Reference semantics (faithful to the source module, bugs included):
    qh = q @ Wq.T + bq            # [B, S, D]
    kh = k @ Wk.T + bk
    vh = v @ Wv.T + bv
    kr = kh.reshape(B, D, S)      # row-major REINTERPRET, not a transpose
    s  = (qh @ kr) * sqrt(D)      # source bug: multiplies by sqrt(D)
    a  = softmax(s, axis=2)
    out = a @ vh                  # [B, S, dk]

Sharding: data-parallel over batch - one batch element per NeuronCore,
8 cores, no collectives. Each core runs the same NEFF with its own shard.

Per-core dataflow (v2 - stage-pipelined projections):
  - q,k and Wq,Wk are PE-transposed in fp32r (scores need tf32 accuracy:
    the kept source bug multiplies scores by sqrt(D), making the softmax
    near-one-hot and argmax-sensitive; bf16 q/k fails the error budget).
  - The v path is bf16 end-to-end: v tiles are cast f32->bf16 on GpSimd
    (an otherwise idle engine) and transposed by the DMA XBAR in one
    [128,2048]->[128,16,128] shot per 4-tile group; Wv likewise (cast +
    one XBAR shot) - no PE transposes and no PSUM->SBUF staging copies
    on the whole v path.  vh only feeds the bf16 a@vh matmul, and out
    is a near-one-hot selection of vh rows, so bf16 projection inputs
    cost ~0.2% extra error against a 2e-2 budget.
  - All projection loops are software-pipelined one stage deep: the
    transposes (+DMA) of stage n+1 are emitted BEFORE the matmuls of
    stage n, so the PE-transpose -> ScalarE-staging-copy -> matmul
    chain's copy latency hides under the previous stage's matmuls.
    (The v1 kernel serialized T(n)->copy(n)->M(n) per iteration and
    lost ~300-500ns of PE time per iteration waiting on ScalarE.)
  - DMA head order: Wk tiles first (they gate the Wk transposes that
    gate everything), then bkb, then two k tile-groups; Wv/bvb are
    issued from inside kh stage 3, Wq/bqT from inside vh stage 2, so
    each weight lands ~2 stages before its consumers without delaying
    the k/v tile stream.  (v1 put 1MB of k tiles ahead of Wk and the
    first real PE work started at 14.6us; now ~6us.)
  - kr = kh.reshape(D, S) is produced with no data movement: the kh
    projection picks its stationary columns with a stride-4 AP packing
    done by the PSUM->SBUF staging copy (ScalarE), which permutes the
    PSUM partition order into kr's layout; the bias-add writes straight
    into the resident kr tile.
  - scores: s[i_part, j] accumulated in PSUM with fp32r (tf32) matmuls.
  - softmax: each 512-chunk PSUM is copied to SBUF (DVE) immediately so
    the bank frees fast; ONE ScalarE exp covers the whole 2048-wide row
    and its accumulator read IS the softmax denominator.
  - probs are transposed by the DMA XBAR (one [128,2048]->[128,16,128]
    shot per i-block) on the ACT HWDGE ring (the SP ring strands the
    transpose's tail descriptors until the next SP dispatch -> ~50%
    stale data).  The attention loop is software-pipelined 4 deep; the
    last two blocks transpose probs on the (then-idle) PE instead.
  - qh bias rides a DVE tensor_scalar_add (ScalarE in the qh phase is
    loaded with the qTc staging copies; DVE is idle there).
  - dummy bf16 matmuls at kernel start fill the initial DMA wait and
    warm the PE clock gate to 2.4 GHz before real work arrives.

Known dead ends (measured, do not retry): v-path PER-TILE XBAR (16
dispatches x ~1.1us fixed cost on ScalarE dwarf the PE saving - batch
4 tiles per shot instead); io bufs < 8; tensor_tensor_reduce with a
PSUM input (wedges the device); per-chunk XBAR splits; PE-transposing
block 13's probs; fp8/DoubleRow (accuracy: near-one-hot softmax is
argmax-sensitive, fp8 value paths exceed the 2e-2 budget); fp32 XBAR
transpose (HW caps 4-byte transposes at 64 output partitions).
"""

from contextlib import ExitStack

import numpy as np

import concourse.bacc as bacc
import concourse.mybir as mybir
import concourse.tile as tile
from concourse.bass_utils import run_bass_kernel_spmd
from concourse.masks import make_identity

AF = mybir.ActivationFunctionType
ALU = mybir.AluOpType
AX = mybir.AxisListType
F32 = mybir.dt.float32
F32R = mybir.dt.float32r
BF16 = mybir.dt.bfloat16

B, S, D = 8, 2048, 512
P = 128
NT_S = S // P          # 16 s-tiles (also j-tiles / i-blocks)
NT_D = D // P          # 4 d-tiles (also e-tiles)
NCH = S // 512         # 4 512-wide chunks of the sequence dim
SQRT_D = float(np.sqrt(np.float32(D)))

N_WARM = 12            # PE warm-up matmuls (fill DMA head, warm clock)
XBAR_PROBS = True      # probs transpose via DMA XBAR instead of PE


def build_nc():
    nc = bacc.Bacc("TRN2", target_bir_lowering=False, debug=False,
                   enable_asserts=False, num_devices=B)

    q = nc.dram_tensor("q", [S, D], F32, kind="ExternalInput").ap()
    k = nc.dram_tensor("k", [S, D], F32, kind="ExternalInput").ap()
    v = nc.dram_tensor("v", [S, D], F32, kind="ExternalInput").ap()
    Wq = nc.dram_tensor("Wq", [D, D], F32, kind="ExternalInput").ap()
    Wk = nc.dram_tensor("Wk", [D, D], F32, kind="ExternalInput").ap()
    Wv = nc.dram_tensor("Wv", [D, D], F32, kind="ExternalInput").ap()
    bq = nc.dram_tensor("bq", [D], F32, kind="ExternalInput").ap()
    bk = nc.dram_tensor("bk", [D], F32, kind="ExternalInput").ap()
    bv = nc.dram_tensor("bv", [D], F32, kind="ExternalInput").ap()
    out = nc.dram_tensor("out", [S, D], F32, kind="ExternalOutput").ap()

    with tile.TileContext(nc) as tc:
        _build(nc, tc, q, k, v, Wq, Wk, Wv, bq, bk, bv, out)
    nc.compile()
    return nc


def _build(nc, tc, q, k, v, Wq, Wk, Wv, bq, bk, bv, out):
    with ExitStack() as ctx:
        _build_inner(nc, tc, ctx, q, k, v, Wq, Wk, Wv, bq, bk, bv, out)


def _build_inner(nc, tc, ctx, q, k, v, Wq, Wk, Wv, bq, bk, bv, out):
    # ---- pools -------------------------------------------------------
    const = ctx.enter_context(tc.tile_pool(name="const", bufs=1))
    resid = ctx.enter_context(tc.tile_pool(name="resid", bufs=1))
    io = ctx.enter_context(tc.tile_pool(name="io", bufs=8))
    work = ctx.enter_context(tc.tile_pool(name="work", bufs=3))
    stats = ctx.enter_context(tc.tile_pool(name="stats", bufs=4))
    psT = ctx.enter_context(tc.tile_pool(name="psT", bufs=3, space="PSUM"))
    psS = ctx.enter_context(tc.tile_pool(name="psS", bufs=3, space="PSUM"))
    psO = ctx.enter_context(tc.tile_pool(name="psO", bufs=2, space="PSUM"))

    # ---- group DMA machinery -----------------------------------------
    # Every x input is streamed as ONE dma_start per 1MB 4-tile group
    # ([128, 4, 512]) instead of four tile DMAs: the in-order SP
    # sequencer pays one dispatch + at most one buffer-free wait per
    # group (measured v5: 66us of sequencer EVENT_SEMAPHORE waits +
    # 63 dispatches serialized the whole input stream to ~140GB/s).
    # Groups are issued ~2 stages before use, AFTER the previous
    # tenant's readers are emitted, so the ring never blocks long.
    k_t = k.rearrange("(t c p) d -> p t c d", p=P, c=4)
    v_t = v.rearrange("(t p) d -> p t d", p=P)
    q_t = q.rearrange("(c s p) d -> p c s d", p=P, c=NCH)
    grp = {}

    def issue_group(key, src_ap):
        # 4 sub-DMAs per group: a single 1MB DMA instruction is capped
        # at ~80GB/s by the per-instruction descriptor issue rate, but
        # concurrent instructions scale; the 4 sub-DMAs share ONE
        # buffer-free wait (only the first pays it), so the sequencer
        # stays cheap.
        g = io.tile([P, NT_D, D], F32R, tag="xg", name=f"g_{key[0]}{key[1]}",
                    bufs=2)
        for c in range(NT_D):
            nc.sync.dma_start(g[:, c, :], src_ap[:, c, :])
        grp[key] = g

    # ---- head DMA: Wk first (gates the W transposes that gate all of
    # kh), then bkb (needed by kh_M(0)), then two k groups.
    Wn_k = work.tile([P, NT_D, D], F32R, tag="wn", name="Wn_k", bufs=1)
    Wk_r = Wk.rearrange("(t p) d -> p t d", p=P).bitcast(F32R)
    for et in range(NT_D):
        nc.sync.dma_start(Wn_k[:, et, :], Wk_r[:, et, :])
    bkb = const.tile([P, D], F32, name="bkb")
    nc.sync.dma_start(bkb[:], bk[None, :].to_broadcast((P, D)))
    issue_group(("k", 0), k_t[:, 0, :, :].bitcast(F32R))
    issue_group(("k", 1), k_t[:, 1, :, :].bitcast(F32R))

    # ---- PE warm-up: dummy matmuls fill the initial DMA wait and bring
    # the HAM clock to 2.4 GHz before real work arrives (values unused).
    warm = const.tile([P, D], BF16, name="warm")
    nc.gpsimd.memset(warm[:], 0.0)
    wps = psO.tile([P, D], F32, tag="o512", name="warm_ps")
    for _ in range(N_WARM):
        nc.tensor.matmul(wps[:], warm[:, :P], warm[:], start=True, stop=True)

    # ---- constants ---------------------------------------------------
    ident_f32 = const.tile([P, P], F32, name="ident_f32")
    make_identity(nc, ident_f32)
    ident_f32r_t = const.tile([P, P], F32R, name="ident_f32r_t")
    nc.vector.tensor_copy(ident_f32r_t[:], ident_f32[:])
    ident_f32r = ident_f32r_t[:]
    ident_bf16 = const.tile([P, P], BF16, name="ident_bf16")
    make_identity(nc, ident_bf16)

    # ---- residents ---------------------------------------------------
    qhT = resid.tile([P, NT_D, S], F32R, tag="qhT", name="qhT")      # [e, i]
    vh = resid.tile([P, NT_S, D], BF16, tag="vh", name="vh")         # [j, e]
    kr = resid.tile([P, NT_D, S], F32R, tag="kr", name="kr")         # [e', j]
    WTv = resid.tile([P, NT_S, P], BF16, tag="WTv", name="WTv")      # xbar'd
    # [d_part, t, e'] view of WTv: block t*4+dt holds Wv[t-block, dt-block]^T
    WTv_de = WTv[:].rearrange("p (t d) c -> p d t c", d=NT_D)
    WT = {}

    # late-issued DMA tiles/constants (filled by the stages below)
    bvb = const.tile([P, D], F32, name="bvb")
    bqT = const.tile([P, NT_D], F32, name="bqT")

    # ================= projection stage pipeline ======================
    # Each stage is (t_fn, m_fn).  Executed with one stage of lookahead:
    # T(0) T(1) M(0) T(2) M(1) ... so stage n's staging copies hide
    # under stage n+1's transposes plus stage n-1's matmuls.

    def wload_T(name, Wn_src, ets=None):
        def f():
            Wn = Wn_src()
            if name not in WT:
                WT[name] = resid.tile([P, NT_D, D], F32R, tag=f"WT_{name}",
                                      name=f"WT_{name}")
            WTt = WT[name]
            for et in (range(NT_D) if ets is None else ets):
                ps = psT.tile([P, NT_D, P], F32R, tag="ps128", name="wt_ps")
                for dt in range(NT_D):
                    nc.tensor.transpose(ps[:, dt, :],
                                        Wn[:, et, dt * P:(dt + 1) * P],
                                        ident_f32r)
                nc.scalar.copy(WTt[:, :, et * P:(et + 1) * P], ps[:])
        return f

    # ---- kh stages: transpose k tiles (PE, fp32r) with a stride-4
    # packed staging copy, then project with WT_k as the moving operand
    # straight into kr's reshape layout.
    kh_state = {}

    def kh_T(t2, post=None):
        def f():
            xTg = work.tile([P, NT_D, 512], F32R, tag="qTc", name="kTg",
                            bufs=2)
            xTg_p = xTg[:].rearrange("p d (a u) -> p d a u", a=4)
            kg = grp.pop(("k", t2))
            for c in range(4):
                ps = psT.tile([P, NT_D, P], F32R, tag="ps128", name="kt_ps")
                for dt in range(NT_D):
                    nc.tensor.transpose(ps[:, dt, :],
                                        kg[:, c, dt * P:(dt + 1) * P],
                                        ident_f32r)
                # packed write: column x of ps goes to (a=x%4, u=32c+x//4)
                nc.scalar.copy(xTg_p[:, :, :, 32 * c:32 * c + 32],
                               ps[:].rearrange("p d (u a) -> p d a u", a=4))
            kh_state[t2] = xTg
            if post is not None:
                post()
        return f

    def kh_M(t2):
        def f():
            xTg = kh_state.pop(t2)
            for a in range(4):
                pp = psS.tile([P, D], F32, tag="ps512", name="kh_ps")
                for dt in range(NT_D):
                    nc.tensor.matmul(pp[:], xTg[:, dt, a * P:(a + 1) * P],
                                     WT["k"][:, dt, :],
                                     start=(dt == 0), stop=(dt == NT_D - 1))
                nc.vector.tensor_tensor(kr[:, t2, a * 512:(a + 1) * 512],
                                        pp[:], bkb[:], op=ALU.add)
        return f

    # ---- v path: bf16 + DMA XBAR, no PE work in the T stages.
    # Casts are split between ScalarE and DVE per group (GpSimd takes
    # 2.1us per [128,512] cast - measured, 4x slower than DVE - and
    # single-handedly starved the PE in the v2 kernel).
    vh_state = {}
    wv_state = {}

    def wv_T():
        # cast Wv f32->bf16 in one 2048-wide DVE op, then XBAR it.
        # The bf16 buffer borrows the vTx tag (same 4KB/partition
        # footprint) so the v-group cast buffers never contend with it.
        def f():
            wbf = work.tile([P, NT_D, D], BF16, tag="vTx", name="Wv_bf",
                            bufs=2)
            nc.vector.tensor_copy(wbf[:], wv_state["wn"][:].bitcast(F32))
            nc.scalar.dma_start(WTv[:], wbf[:], transpose=True)
        return f

    def issue_wv_dma():
        wn = work.tile([P, NT_D, D], F32R, tag="wn", name="Wn_v", bufs=1)
        Wv_r = Wv.rearrange("(t p) d -> p t d", p=P).bitcast(F32R)
        for et in range(NT_D):
            nc.sync.dma_start(wn[:, et, :], Wv_r[:, et, :])
        nc.sync.dma_start(bvb[:], bv[None, :].to_broadcast((P, D)))
        wv_state["wn"] = wn

    def vh_T(g, post=None):
        def f():
            vbf = work.tile([P, NT_D, D], BF16, tag="vbf", name="vbf",
                            bufs=2)
            vg = grp.pop(("v", g))
            # one 1024-wide cast on ScalarE, one on DVE
            nc.scalar.copy(vbf[:, 0:2, :], vg[:, 0:2, :].bitcast(F32))
            nc.vector.tensor_copy(vbf[:, 2:4, :], vg[:, 2:4, :].bitcast(F32))
            vTx = work.tile([P, NT_S, P], BF16, tag="vTx", name="vTx",
                            bufs=2)
            nc.scalar.dma_start(vTx[:], vbf[:], transpose=True)
            vh_state[g] = vTx
            if post is not None:
                post()
        return f

    def vh_M(g):
        def f():
            vTx = vh_state.pop(g)
            for c in range(4):
                st = 4 * g + c
                pp = psS.tile([P, D], F32, tag="ps512", name="vh_ps")
                for dt in range(NT_D):
                    nc.tensor.matmul(pp[:], vTx[:, 4 * c + dt, :],
                                     WTv_de[:, dt, :, :],
                                     start=(dt == 0), stop=(dt == NT_D - 1))
                nc.vector.tensor_tensor(vh[:, st, :], pp[:], bvb[:],
                                        op=ALU.add)
        return f

    def issue_wq_dma():
        # (ACT-ring input DMAs corrupt data when mixed with the XBAR
        # transposes on that ring - measured rel err 180; keep ALL
        # plain input DMAs on the SP ring.)
        wn = work.tile([P, NT_D, D], F32R, tag="wn", name="Wn_q", bufs=1)
        Wq_r = Wq.rearrange("(t p) d -> p t d", p=P).bitcast(F32R)
        for et in range(NT_D):
            nc.sync.dma_start(wn[:, et, :], Wq_r[:, et, :])
        nc.sync.dma_start(bqT[:], bq.rearrange("(t p) -> p t", p=P))
        wq_state["wn"] = wn

    wq_state = {}

    # ---- qh stages: PE-transpose q (fp32r), project with WT_q
    # stationary into qhT [e_part, i]; bias rides DVE (ScalarE is busy
    # with the qTc staging copies in this phase).
    qh_state = {}

    def qh_T(ic, post=None):
        def f():
            qTc = work.tile([P, NT_D, 512], F32R, tag="qTc", name="qTc",
                            bufs=2)
            qg = grp.pop(("q", ic))
            for c in range(4):
                ps = psT.tile([P, NT_D, P], F32R, tag="ps128", name="qt_ps")
                for dt in range(NT_D):
                    nc.tensor.transpose(ps[:, dt, :],
                                        qg[:, c, dt * P:(dt + 1) * P],
                                        ident_f32r)
                nc.scalar.copy(qTc[:, :, c * P:(c + 1) * P], ps[:])
            qh_state[ic] = qTc
            if post is not None:
                post()
        return f

    def qh_M(ic):
        def f():
            qTc = qh_state.pop(ic)
            for et in range(NT_D):
                pp = psS.tile([P, 512], F32, tag="ps512", name="qh_ps")
                for dt in range(NT_D):
                    nc.tensor.matmul(pp[:],
                                     WT["q"][:, dt, et * P:(et + 1) * P],
                                     qTc[:, dt, :],
                                     start=(dt == 0), stop=(dt == NT_D - 1))
                nc.vector.tensor_scalar_add(
                    qhT[:, et, ic * 512:(ic + 1) * 512], pp[:],
                    bqT[:, et:et + 1])
        return f

    # ---- stage schedule (T-column, M-column) -------------------------
    # Wv/Wq DMAs are issued early from inside kh stages (the ring is
    # in-order; each lands ~2+ stages before its consumers).  The Wv
    # bf16 cast rides DVE in two chunks during kh; the Wq transposes
    # fill the PE hole left by vh_T(1) (which has no PE work).
    def seq(*fns):
        def f():
            for g in fns:
                g()
        return f

    # ---- stage schedule ----------------------------------------------
    # The input stream is HBM-limited (~150GB/s per core measured; the
    # SBUF<->SBUF XBAR streams hit 330GB/s+ on the same engines, so the
    # wall is the DRAM side), i.e. ~95us for the 15.7MB of inputs.  The
    # stream order is therefore k -> q0 -> v -> q1..q3, and everything
    # after kr+qh(0) is spliced INTO the attention pipeline: scores only
    # need kr and their own qh chunk, and the first a@vh runs 4 blocks
    # in, so the v stream and the q tail hide under early score blocks.
    #
    # Wn lifetimes through the single "wn" buffer are DISJOINT: Wk
    # (head -> S0), Wq (issued S1, transposed S3/S4), Wv (issued S4
    # after Wq's last readers are emitted).  An early-DMA'd-but-late-
    # consumed weight in this buffer blocks the next weight's DMA, and
    # the in-order SP ring then head-of-line-blocks every tile behind
    # it (measured: a ~30us stall of the whole input stream).  Group
    # DMAs are issued as a `post` of the stage that emits the previous
    # buffer tenant's readers - both so the wait is satisfied quickly
    # and so the pool dependency tracking sees those readers.
    wq_src = lambda: wq_state["wn"]  # noqa: E731
    kg = lambda t2: (lambda: issue_group(("k", t2), k_t[:, t2, :, :].bitcast(F32R)))  # noqa: E731,E501
    vg = lambda g: (lambda: issue_group(("v", g), v_t[:, 4 * g:4 * g + 4, :].bitcast(F32R)))  # noqa: E731,E501
    qg = lambda ic: (lambda: issue_group(("q", ic), q_t[:, ic, :, :].bitcast(F32R)))  # noqa: E731,E501
    stages = [
        (wload_T("k", lambda: Wn_k), None),
        (kh_T(0, post=seq(kg(2), issue_wq_dma)), None),
        (kh_T(1, post=kg(3)), kh_M(0)),
        (seq(kh_T(2, post=qg(0)), wload_T("q", wq_src, ets=(0, 1))),
         kh_M(1)),
        (seq(kh_T(3), wload_T("q", wq_src, ets=(2, 3)), issue_wv_dma,
             vg(0)), kh_M(2)),
        (qh_T(0, post=vg(1)), kh_M(3)),
        (None, qh_M(0)),
    ]
    for t_fn, m_fn in stages:
        if t_fn is not None:
            t_fn()
        if m_fn is not None:
            m_fn()

    # ---- attention per 128-row i-block -------------------------------
    def scores_softmax(ib):
        mx = stats.tile([P, NCH], F32, tag="mx", name="mx")
        p_bf = work.tile([P, S], BF16, tag="p", name="p_bf", bufs=2)

        s_sb = work.tile([P, NCH, 512], F32, tag="s_sb", name="s_sb", bufs=2)
        for jc in range(NCH):
            sp = psS.tile([P, 512], F32, tag="ps512", name="s_ps")
            for et in range(NT_D):
                nc.tensor.matmul(sp[:],
                                 qhT[:, et, ib * P:(ib + 1) * P],
                                 kr[:, et, jc * 512:(jc + 1) * 512],
                                 start=(et == 0), stop=(et == NT_D - 1))
            # fused PSUM->SBUF stage + row-max pair: the copy frees the
            # PSUM bank fast (it is the bank's only reader), the max runs
            # on the SBUF copy.  (tensor_tensor_reduce from PSUM wedges
            # the HW - keep the plain copy + max pair.)
            nc.vector.tensor_copy(s_sb[:, jc, :], sp[:])
            nc.vector.reduce_max(mx[:, jc:jc + 1], s_sb[:, jc, :],
                                 axis=AX.X)

        gmx = stats.tile([P, 1], F32, tag="gmx", name="gmx")
        ngmx = stats.tile([P, 1], F32, tag="ngmx", name="ngmx")
        den = stats.tile([P, 1], F32, tag="den", name="den")
        rs = stats.tile([P, 1], F32, tag="rs", name="rs")
        nc.vector.reduce_max(gmx[:], mx[:], axis=AX.X)
        nc.vector.tensor_scalar_mul(ngmx[:], gmx[:], -SQRT_D)

        # one exp over the whole 2048-wide row; the accumulator read IS
        # the softmax denominator.
        nc.scalar.activation(p_bf[:], s_sb[:], AF.Exp, bias=ngmx[:, 0:1],
                             scale=SQRT_D, accum_out=den[:])
        nc.vector.reciprocal(rs[:], den[:])

        pT = work.tile([P, NT_S, P], BF16, tag="pT", name="pT", bufs=4)
        # Last two blocks: no further scores hide the XBAR dispatch+stream
        # chain at pipeline drain, but the PE is idling there - PE
        # transposes are faster for exactly those blocks.
        if XBAR_PROBS and ib < NT_S - 2:
            nc.scalar.dma_start(pT[:], p_bf[:], transpose=True)
        else:
            for a in range(NCH):
                ps = psT.tile([P, NT_D, P], BF16, tag="ps128", name="pt_ps")
                for b2 in range(NT_D):
                    jt = a * NT_D + b2
                    nc.tensor.transpose(ps[:, b2, :],
                                        p_bf[:, jt * P:(jt + 1) * P],
                                        ident_bf16[:])
                nc.vector.tensor_copy(pT[:, a * NT_D:(a + 1) * NT_D, :], ps[:])
        return pT, rs

    def attend_out(ib, pT, rs):
        op = psO.tile([P, D], F32, tag="o512", name="o_ps")
        for jt in range(NT_S):
            nc.tensor.matmul(op[:], pT[:, jt, :], vh[:, jt, :],
                             start=(jt == 0), stop=(jt == NT_S - 1))
        # 1/den scale on DVE (keeps ScalarE free for the exp/XBAR chain;
        # a fused scale+bias scalar_tensor_tensor here costs 900ns vs
        # 430ns - 2-input DVE ops are 2x - and DVE is the block's
        # critical path: measured +20us on the attention phase)
        o_sb = work.tile([P, D], F32, tag="o_sb", name="o_sb", bufs=2)
        nc.vector.tensor_scalar_mul(o_sb[:], op[:], rs[:, 0:1])
        nc.sync.dma_start(out.rearrange("(t p) e -> p t e", p=P)[:, ib, :],
                          o_sb[:])

    # 4-deep: the scores->exp->XBAR->a@vh chain spans ~2 block periods;
    # three blocks of scores/softmax ahead of each a@vh absorbs the
    # chain's jitter (exp/dispatch variance) entirely.  The v path and
    # the late qh chunks are spliced between the early blocks: the
    # first a@vh (which needs full vh) only runs after scores(3), by
    # which time the v stream has landed; scores(4c..) only need qh
    # chunk c, emitted just ahead of them while the q tail streams.
    pend = [scores_softmax(0)]
    wv_T()()
    vh_T(0, post=vg(2))()
    pend.append(scores_softmax(1))
    vh_M(0)()
    vh_T(1, post=vg(3))()
    pend.append(scores_softmax(2))
    vh_M(1)()
    vh_T(2, post=qg(1))()
    pend.append(scores_softmax(3))
    vh_M(2)()
    vh_T(3, post=qg(2))()
    vh_M(3)()
    attend_out(0, *pend.pop(0))
    qh_T(1, post=qg(3))()
    qh_M(1)()
    for ib in range(1, NT_S):
        if ib + 3 < NT_S:
            pend.append(scores_softmax(ib + 3))
        if ib == 4:
            qh_T(2)()
            qh_M(2)()
        elif ib == 8:
            qh_T(3)()
            qh_M(3)()
        attend_out(ib, *pend.pop(0))


def _ensure_axon_hooks_module():
    """antenv.axon_hooks is missing on this image; provide it (with the real
    ctypes NTFF hook when available) so run_bass_kernel_spmd(trace=True)
    degrades gracefully instead of raising ImportError."""
    import sys
    import types
    try:
        import antenv
        import antenv.axon_hooks  # noqa: F401
        return
    except ImportError:
        pass
    try:
        mod = types.ModuleType("antenv.axon_hooks")
        state = {"hook": None}
        mod.set_axon_ntff_profile_hook = lambda h: state.__setitem__("hook", h)
        mod.get_axon_ntff_profile_hook = lambda: state["hook"]
        sys.modules["antenv.axon_hooks"] = mod
        antenv.axon_hooks = mod
        try:
            if "/root/.axon_site" not in sys.path:
                sys.path.insert(0, "/root/.axon_site")
            from trn_agent_boot.trn_boot import _ntff_profile_via_ctypes

            mod.set_axon_ntff_profile_hook(
                _ntff_profile_via_ctypes("/opt/axon/libaxon_pjrt.so")
            )
        except Exception:
            pass
    except Exception:
        pass


_ensure_axon_hooks_module()

_NC_CACHE = None


def _get_nc():
    global _NC_CACHE
    if _NC_CACHE is None:
        _NC_CACHE = build_nc()
    return _NC_CACHE


def kernel(q, k, v, Wq, bq, Wk, bk, Wv, bv):
    nc = _get_nc()
    in_maps = []
    for b in range(B):
        in_maps.append({
            "q": np.ascontiguousarray(q[b], dtype=np.float32),
            "k": np.ascontiguousarray(k[b], dtype=np.float32),
            "v": np.ascontiguousarray(v[b], dtype=np.float32),
            "Wq": np.ascontiguousarray(Wq, dtype=np.float32),
            "Wk": np.ascontiguousarray(Wk, dtype=np.float32),
            "Wv": np.ascontiguousarray(Wv, dtype=np.float32),
            "bq": np.ascontiguousarray(bq, dtype=np.float32),
            "bk": np.ascontiguousarray(bk, dtype=np.float32),
            "bv": np.ascontiguousarray(bv, dtype=np.float32),
        })
    res = run_bass_kernel_spmd(nc, in_maps, core_ids=list(range(B)))
    return np.stack([res.results[b]["out"] for b in range(B)], axis=0)
